# revision 50
# baseline (speedup 1.0000x reference)
"""Parametrized Bass/Tile transformer-block kernel for TRN2, 8-core SPMD.

Sharding: core c -> (batch b=c//2, query parity p=c%2). Each core computes the
output rows for tokens {2t+p} of its batch element. Fully uniform program, no
collectives; causal structure is uniform because local q-block i attends
exactly 2i+2 key blocks on every core (parity handled by a fixed mask).

v2 layout: single fused dataflow region. Q projections first (frees h1qt for
the streamed-W1 buffer), K/V projections interleaved between attention
superblocks so the PE stays fed while ACT chews softmax exps, Wo+LN2 per
512-token group as soon as its attention superblocks land, FFN streamed
per-group afterwards (W1/W2 chunks DMA'd just-in-time into space freed by
stage-A pools). Attention operands (q/k/v) are fp8e4m3 in SBUF; exps are
computed on [P,1024] PSUM spans (2 banks) to halve ACT instruction count.
LN2's rstd uses exp(-0.5*ln(var+eps)) so the whole pre-FFN phase stays on the
natural_log_exp ACT table set; gelus are dep-ordered after the last exp so
there is exactly one table switch.
"""
import sys
for _p in ("/opt/trn_rl_repo", "/root/.axon_site/_ro/trn_rl_repo"):
    if _p not in sys.path:
        sys.path.append(_p)

from contextlib import ExitStack

import numpy as np
import ml_dtypes

import concourse.bass as bass
import concourse.mybir as mybir
import concourse.tile as tile
from concourse import bacc
from concourse.bass import ts, ds
from concourse.tile_rust import add_dep_helper

AF = mybir.ActivationFunctionType
DT = mybir.dt
BF = ml_dtypes.bfloat16
P = 128
EPS = 1e-5


def build_program(D, T, H, DK, FF, reps=1):
    Tq = T // 2
    NCD = D // P            # feature chunks
    NCF = FF // P           # hidden chunks
    NHP = H // 2            # head pairs
    NQG = Tq // 512         # 512-token groups over own queries
    NM = Tq // 256          # attention q superblocks (256 wide)
    NKB = T // P            # key blocks
    WV = min(512, D)        # V-projection column group width
    NVG = D // WV
    HPG = WV // DK          # heads per V col group
    F8 = DT.float8e4
    assert D == H * DK and Tq % 512 == 0 and NHP == NCD

    nc = bacc.Bacc(None, target_bir_lowering=False)

    # ---- DRAM I/O ----
    h1t_d = nc.dram_tensor("h1t", [D, T], DT.bfloat16, kind="ExternalInput")
    h1qt_d = nc.dram_tensor("h1qt", [D, Tq], DT.bfloat16, kind="ExternalInput")
    xqt_d = nc.dram_tensor("xqt", [D, Tq], DT.float32, kind="ExternalInput")
    wq_d = nc.dram_tensor("wq", [D, D], DT.bfloat16, kind="ExternalInput")
    wk_d = nc.dram_tensor("wk", [D, D], DT.bfloat16, kind="ExternalInput")
    wv_d = nc.dram_tensor("wv", [D, D], DT.bfloat16, kind="ExternalInput")
    wo_d = nc.dram_tensor("wo", [D, D], DT.bfloat16, kind="ExternalInput")
    w1_d = nc.dram_tensor("w1", [D, FF], DT.bfloat16, kind="ExternalInput")
    w2_d = nc.dram_tensor("w2", [FF, D], DT.bfloat16, kind="ExternalInput")
    b1c_d = nc.dram_tensor("b1c", [P, NCF], DT.float32, kind="ExternalInput")
    mask_d = nc.dram_tensor("maskc", [P, 2, 512], DT.bfloat16, kind="ExternalInput")
    ident_d = nc.dram_tensor("ident", [P, P], DT.bfloat16, kind="ExternalInput")
    outt_d = nc.dram_tensor("outt", [D, Tq], DT.float32, kind="ExternalOutput")

    h1t_r = h1t_d[:].rearrange("(c p) t -> p c t", p=P)
    h1qt_r = h1qt_d[:].rearrange("(c p) t -> p c t", p=P)
    xqt_r = xqt_d[:].rearrange("(c p) t -> p c t", p=P)
    wq_r = wq_d[:].rearrange("(c p) n -> p c n", p=P)
    wk_r = wk_d[:].rearrange("(c p) n -> p c n", p=P)
    wv_r = wv_d[:].rearrange("(c p) n -> p c n", p=P)
    wo_r = wo_d[:].rearrange("(c p) n -> p c n", p=P)
    w1_r = w1_d[:].rearrange("(c p) n -> p c n", p=P)
    w2_r = w2_d[:].rearrange("(c p) n -> p c n", p=P)
    mask_flat = mask_d[:].rearrange("p a b -> p (a b)")
    outt_r = outt_d[:].rearrange("(c p) t -> p c t", p=P)

    with tile.TileContext(nc) as tc, ExitStack() as top:
        constp = top.enter_context(tc.tile_pool(name="const", bufs=1))
        mask_sb = constp.tile([P, 1024], DT.bfloat16)
        nc.scalar.dma_start(mask_sb[:], mask_flat)
        ident_sb = constp.tile([P, P], DT.bfloat16)
        nc.scalar.dma_start(ident_sb[:], ident_d[:])
        # additive causal mask: 0 where allowed, -30000 where masked
        maskm = constp.tile([P, 1024], DT.bfloat16)
        nc.scalar.activation(maskm[:], mask_sb[:], AF.Copy,
                             bias=-30000.0, scale=30000.0)
        b1c_sb = constp.tile([P, NCF], DT.float32)
        nc.scalar.dma_start(b1c_sb[:], b1c_d[:])
        ones_bf = constp.tile([P, 1], DT.bfloat16)
        nc.vector.memset(ones_bf[:], 1.0)
        ones_f = constp.tile([1, P], DT.float32)
        nc.vector.memset(ones_f[:], 1.0)
        ones_r = constp.tile([1, P], DT.float32r)
        with nc.allow_low_precision(reason="f32r ones"):
            nc.vector.tensor_copy(ones_r[:], ones_f[:])
        zero_b = constp.tile([P, 1], DT.float32)
        nc.vector.memset(zero_b[:], 0.0)
        zero1 = constp.tile([1, 1], DT.float32)
        nc.vector.memset(zero1[:], 0.0)
        eps_b = constp.tile([1, 1], DT.float32)
        nc.vector.memset(eps_b[:], EPS)

        for _rep in range(reps):
            # ---------------- pools ----------------
            qkv_cm = tc.tile_pool(name="p_qkv", bufs=1, side="left")
            p_qkv = qkv_cm.__enter__()
            qt_sb = p_qkv.tile([P, NHP, Tq], F8)
            kt_sb = p_qkv.tile([P, NHP, T], F8)
            v_sb = p_qkv.tile([P, NKB, 65 * H], F8)
            a_cm = tc.tile_pool(name="p_a", bufs=1, side="left")
            p_a = a_cm.__enter__()
            wv_sb = p_a.tile([P, NCD, D], DT.bfloat16)
            h1s_cm = tc.tile_pool(name="p_h1s", bufs=2, side="left")
            p_h1s = h1s_cm.__enter__()
            aq_cm = tc.tile_pool(name="p_aq", bufs=1, side="left")
            p_aq = aq_cm.__enter__()
            h1qt_sb = p_aq.tile([P, NCD, Tq], DT.bfloat16)

            rep_cm = ExitStack()
            w_a = rep_cm.enter_context(
                tc.tile_pool(name="w_a", bufs=3, side="right"))
            p_wor = rep_cm.enter_context(
                tc.tile_pool(name="p_wor", bufs=1, side="right"))
            wo_sb = p_wor.tile([P, NCD, D], DT.bfloat16)
            p_oxx = rep_cm.enter_context(
                tc.tile_pool(name="p_oxx", bufs=1, side="right"))
            ot_sb = p_oxx.tile([P, NCD, Tq], DT.bfloat16)
            x1_sb = p_oxx.tile([P, NCD, Tq], DT.bfloat16)
            xh_sb = p_oxx.tile([P, NCD, Tq], DT.bfloat16)
            p_exp = rep_cm.enter_context(
                tc.tile_pool(name="p_exp", bufs=2, side="right"))
            p_nrm = rep_cm.enter_context(
                tc.tile_pool(name="p_nrm", bufs=1, side="right"))
            p_sq = rep_cm.enter_context(
                tc.tile_pool(name="p_sq", bufs=1, side="right"))
            p_st = rep_cm.enter_context(
                tc.tile_pool(name="p_st", bufs=1, side="right"))
            p_xq = rep_cm.enter_context(
                tc.tile_pool(name="p_xq", bufs=2, side="right"))
            p_out = rep_cm.enter_context(
                tc.tile_pool(name="p_out", bufs=2, side="right"))
            ps_s = rep_cm.enter_context(
                tc.tile_pool(name="ps_s", bufs=2, space="PSUM"))
            ps_av = rep_cm.enter_context(
                tc.tile_pool(name="ps_av", bufs=2, space="PSUM"))
            ps_main = rep_cm.enter_context(
                tc.tile_pool(name="ps_m", bufs=2, space="PSUM"))

            # ---------------- startup DMAs ----------------
            for c in range(NCD):
                nc.gpsimd.dma_start(
                    h1qt_sb[:, c, ds(0, 512)], h1qt_r[:, c, ds(0, 512)])
            for c in range(NCD):
                nc.gpsimd.dma_start(
                    h1qt_sb[:, c, ds(512, 512)], h1qt_r[:, c, ds(512, 512)])
            for c in range(NCD):
                nc.scalar.dma_start(wv_sb[:, c], wv_r[:, c])
            for c in range(NCD):
                nc.scalar.dma_start(wo_sb[:, c], wo_r[:, c])
            for h in range(H):
                nc.vector.memset(v_sb[:, :, ds(65 * h + 64, 1)], 1.0)

            def h1_load(g):
                h1g = p_h1s.tile([P, NCD, 512], DT.bfloat16, tag="h1g")
                for c in range(NCD):
                    nc.gpsimd.dma_start(
                        h1g[:, c], h1t_r[:, c, ds(512 * g, 512)])
                return h1g

            # ---------------- stage A pieces ----------------
            def q_proj():
                for g in range(NQG):
                    for hp in range(NHP):
                        wq_t = w_a.tile([P, NCD, P], DT.bfloat16, tag="wqk")
                        nc.sync.dma_start(wq_t[:], wq_r[:, :, ds(P * hp, P)])
                        ps_q = ps_main.tile([P, 512], DT.float32, tag="ps")
                        for c in range(NCD):
                            nc.tensor.matmul(
                                ps_q[:], wq_t[:, c],
                                h1qt_sb[:, c, ds(512 * g, 512)],
                                start=(c == 0), stop=(c == NCD - 1))
                        with nc.allow_low_precision(reason="fp8 attention operand"):
                            nc.vector.tensor_copy(
                                qt_sb[:, hp, ds(512 * g, 512)], ps_q[:])

            def k_unit(g, h1g, hp):
                wk_t = w_a.tile([P, NCD, P], DT.bfloat16, tag="wqk")
                nc.sync.dma_start(wk_t[:], wk_r[:, :, ds(P * hp, P)])
                ps_k = ps_main.tile([P, 512], DT.float32, tag="ps")
                for c in range(NCD):
                    nc.tensor.matmul(
                        ps_k[:], wk_t[:, c], h1g[:, c],
                        start=(c == 0), stop=(c == NCD - 1))
                with nc.allow_low_precision(reason="fp8 attention operand"):
                    nc.vector.tensor_copy(kt_sb[:, hp, ds(512 * g, 512)], ps_k[:])

            def v_unit(tb, g2, h1g):
                ps_v = ps_main.tile([P, WV], DT.float32, tag="ps")
                for c in range(NCD):
                    nc.tensor.matmul(
                        ps_v[:], h1g[:, c, ds(P * (tb % 4), P)],
                        wv_sb[:, c, ds(WV * g2, WV)],
                        start=(c == 0), stop=(c == NCD - 1))
                with nc.allow_low_precision(reason="fp8 attention operand"):
                    for hh in range(HPG):
                        h = HPG * g2 + hh
                        nc.vector.tensor_copy(
                            v_sb[:, tb, ds(65 * h, 64)],
                            ps_v[:, ds(64 * hh, 64)])

            def k_units(g, h1g):
                return [lambda hp=hp: k_unit(g, h1g, hp) for hp in range(NHP)]

            def v_units(tbs, h1g):
                return [lambda tb=tb, g2=g2: v_unit(tb, g2, h1g)
                        for tb in tbs for g2 in range(NVG)]

            # ---------------- attention superblock ----------------
            def attn_norm(mp, hp, po):
                rec = p_nrm.tile([1, 512], DT.float32r, tag="rec")
                with nc.allow_low_precision(reason="f32r broadcast operand"):
                    nc.vector.reciprocal(rec[:], po[64:65, :])
                pbc = ps_main.tile([P, 512], DT.float32, tag="ps", name="pbc")
                nc.tensor.matmul(pbc[0:64, :], ones_r[:, 0:64], rec[:],
                                 start=True, stop=True)
                rb = p_nrm.tile([64, 512], DT.float32, tag="rb")
                nc.vector.tensor_copy(rb[:], pbc[0:64, :])
                for hi in range(2):
                    nc.vector.tensor_mul(
                        ot_sb[64 * hi:64 * hi + 64, hp, ds(256 * mp, 256)],
                        po[0:64, ds(256 * hi, 256)], rb[:, ds(256 * hi, 256)])

            def attn(mp, fillers=()):
                nkv = 4 * mp + 4
                ngrp = nkv // 4
                scale = float(DK) ** -0.5
                L = len(fillers)
                pending = None
                for hp in range(NHP):
                    po = ps_av.tile([P, 512], DT.float32, tag="po")
                    for gk in range(ngrp):
                        band = gk == ngrp - 1
                        pss = [ps_s.tile([P, 1024], DT.float32, tag="ps_s",
                                         name=f"pss{_i}") for _i in range(2)]
                        for j in range(4):
                            kb = 4 * gk + j
                            for hi in range(2):
                                pb = 64 * hi
                                nc.tensor.matmul(
                                    pss[hi][:, ds(256 * j, 256)],
                                    kt_sb[pb:pb + 64, hp, ds(P * kb, P)],
                                    qt_sb[pb:pb + 64, hp, ds(256 * mp, 256)],
                                    start=(j % 2 == 0),
                                    stop=(j % 2 == 1) and not band,
                                    tile_position=(pb, 0),
                                    skip_group_check=True)
                        if band:
                            # fold the causal mask in additively on the PE:
                            # exp(-30000/8) underflows to exactly 0
                            for hi in range(2):
                                for s in range(2):
                                    nc.tensor.matmul(
                                        pss[hi][:, ds(512 * s, 512)],
                                        ident_sb[:],
                                        maskm[:, ds(512 * s, 512)],
                                        start=False, stop=True,
                                        skip_group_check=True)
                        exs = []
                        for hi in range(2):
                            ex = p_exp.tile([P, 2, 2, 256], F8, tag="ex")
                            with nc.allow_low_precision(reason="fp8 softmax wts"):
                                a = nc.scalar.activation(ex[:], pss[hi][:], AF.Exp,
                                                         bias=zero_b[:], scale=scale)
                            attn.last_exp = a.ins if hasattr(a, "ins") else a
                            exs.append(ex)
                        for pp in range(2):
                            for hi in range(2):
                                h = 2 * hp + hi
                                nc.tensor.matmul(
                                    po[0:65, ds(256 * hi, 256)],
                                    v_sb[:, ds(4 * gk + 2 * pp, 2), ds(65 * h, 65)],
                                    exs[hi][:, pp],
                                    start=(gk == 0 and pp == 0 and hi == 0),
                                    stop=(gk == ngrp - 1 and pp == 1 and hi == 1),
                                    perf_mode=mybir.MatmulPerfMode.DoubleRow,
                                    skip_group_check=True)
                    # norm chain runs one hp behind so it never heads the PE queue
                    if pending is not None:
                        attn_norm(mp, pending[0], pending[1])
                    pending = (hp, po)
                    for u in fillers[L * hp // NHP: L * (hp + 1) // NHP]:
                        u()
                attn_norm(mp, pending[0], pending[1])

            # ---------------- Wo + residual per group ----------------
            def c_unit(g, c):
                ps_x = ps_main.tile([P, 512], DT.float32, tag="ps")
                for k in range(NCD):
                    nc.tensor.matmul(
                        ps_x[:], wo_sb[:, k, ds(P * c, P)],
                        ot_sb[:, k, ds(512 * g, 512)],
                        start=(k == 0), stop=(k == NCD - 1))
                xq_t = p_xq.tile([P, 512], DT.float32, tag="xq")
                nc.sync.dma_start(xq_t[:], xqt_r[:, c, ds(512 * g, 512)])
                nc.vector.tensor_add(
                    x1_sb[:, c, ds(512 * g, 512)], ps_x[:], xq_t[:])

            def phase_C(g):
                for c in range(NCD):
                    c_unit(g, c)

            def c_units(g):
                return [lambda c=c: c_unit(g, c) for c in range(NCD)]

            # ---------------- LN2 per group ----------------
            def phase_D(g):
                sl = ds(512 * g, 512)
                ps1 = ps_main.tile([P, 512], DT.float32, tag="ps", name="ps1")
                for c in range(NCD):
                    nc.tensor.matmul(ps1[0:1, :], ones_bf[:], x1_sb[:, c, sl],
                                     start=(c == 0), stop=(c == NCD - 1))
                ps2 = ps_main.tile([P, 512], DT.float32, tag="ps", name="ps2")
                for c in range(NCD):
                    sq = p_sq.tile([P, 512], DT.bfloat16, tag="sq")
                    nc.scalar.activation(sq[:], x1_sb[:, c, sl], AF.Square,
                                         bias=zero_b[:])
                    nc.tensor.matmul(ps2[0:1, :], ones_bf[:], sq[:],
                                     start=(c == 0), stop=(c == NCD - 1))
                mu = p_st.tile([1, 512], DT.float32, tag="mu")
                nc.vector.tensor_scalar_mul(mu[:], ps1[0:1, :], 1.0 / D)
                msq = p_st.tile([1, 512], DT.float32, tag="msq")
                nc.vector.tensor_mul(msq[:], mu[:], mu[:])
                nc.vector.scalar_tensor_tensor(
                    msq[:], ps2[0:1, :], 1.0 / D, msq[:],
                    mybir.AluOpType.mult, mybir.AluOpType.subtract)
                # rstd = exp(-0.5 * ln(var + eps)) — stays on the exp/ln table set
                lnv = p_st.tile([1, 512], DT.float32, tag="lnv")
                nc.scalar.activation(lnv[:], msq[:], AF.Ln, bias=eps_b[:])
                r2f = p_st.tile([1, 512], DT.float32, tag="r2f")
                last_act = nc.scalar.activation(r2f[:], lnv[:], AF.Exp,
                                                bias=zero1[:], scale=-0.5)
                r2 = p_st.tile([1, 512], DT.float32r, tag="r2")
                with nc.allow_low_precision(reason="f32r broadcast operand"):
                    nc.vector.tensor_copy(r2[:], r2f[:])
                m2r = p_st.tile([1, 512], DT.float32r, tag="m2r")
                with nc.allow_low_precision(reason="f32r broadcast operand"):
                    nc.vector.tensor_mul(m2r[:], mu[:], r2[:])
                pb1 = ps_main.tile([P, 512], DT.float32, tag="ps", name="pb1")
                nc.tensor.matmul(pb1[:], ones_r[:], r2[:], start=True, stop=True)
                r2b = p_sq.tile([P, 512], DT.float32, tag="r2b")
                nc.vector.tensor_copy(r2b[:], pb1[:])
                pb2 = ps_main.tile([P, 512], DT.float32, tag="ps", name="pb2")
                nc.tensor.matmul(pb2[:], ones_r[:], m2r[:], start=True, stop=True)
                m2rb = p_sq.tile([P, 512], DT.float32, tag="m2rb")
                nc.vector.tensor_copy(m2rb[:], pb2[:])
                for c in range(NCD):
                    tmp = p_sq.tile([P, 512], DT.float32, tag="tmp")
                    nc.vector.tensor_mul(tmp[:], x1_sb[:, c, sl], r2b[:])
                    nc.vector.tensor_sub(xh_sb[:, c, sl], tmp[:], m2rb[:])
                return last_act.ins if hasattr(last_act, "ins") else last_act

            # ---------------- FFN per group ----------------
            def e_mm_chain(g1g, m, g):
                w1_t = p_w1s.tile([P, NCD, P], DT.bfloat16, tag="w1")
                nc.scalar.dma_start(w1_t[:], w1_r[:, :, ds(P * m, P)])
                ps_f = ps_main.tile([P, 512], DT.float32, tag="ps")
                for c in range(NCD):
                    nc.tensor.matmul(
                        ps_f[:], w1_t[:, c],
                        xh_sb[:, c, ds(512 * g, 512)],
                        start=(c == 0), stop=(c == NCD - 1))
                return ps_f

            def e_stage_units(g1g, g):
                # matmul + bf16 staging copy only; gelu deferred so these can
                # interleave with attention without ACT table thrash
                def unit(m):
                    ps_f = e_mm_chain(g1g, m, g)
                    nc.vector.tensor_copy(g1g[:, m], ps_f[:])
                return [lambda m=m: unit(m) for m in range(NCF)]

            def e_gelu_batch(g1g, dep_inst=None):
                for m in range(NCF):
                    a = nc.scalar.activation(g1g[:, m], g1g[:, m], AF.Gelu,
                                             bias=b1c_sb[:, ds(m, 1)])
                    if dep_inst is not None:
                        # every gelu must wait: the scheduler slots ACT ops by
                        # availability, and a gelu between exps costs two
                        # ~1.3us ACT-table loads
                        add_dep_helper(
                            a.ins if hasattr(a, "ins") else a, dep_inst,
                            sync=False, reason="gelu batch after attn exps")

            def phase_E(g, g1g):
                for m in range(NCF):
                    ps_f = e_mm_chain(g1g, m, g)
                    nc.scalar.activation(g1g[:, m], ps_f[:], AF.Gelu,
                                         bias=b1c_sb[:, ds(m, 1)])

            def phase_F(g, g1g, p_w2s):
                for c in range(NCD):
                    w2_t = p_w2s.tile([P, NCF, P], DT.bfloat16, tag="w2")
                    nc.sync.dma_start(w2_t[:], w2_r[:, :, ds(P * c, P)])
                    pf = ps_main.tile([P, 512], DT.float32, tag="ps")
                    for hh in range(NCF):
                        nc.tensor.matmul(pf[:], w2_t[:, hh], g1g[:, hh],
                                         start=(hh == 0), stop=(hh == NCF - 1))
                    out_t = p_out.tile([P, 512], DT.float32, tag="out")
                    nc.vector.tensor_add(out_t[:], pf[:],
                                         x1_sb[:, c, ds(512 * g, 512)])
                    nc.gpsimd.dma_start(outt_r[:, c, ds(512 * g, 512)], out_t[:])

            # ---------------- emission schedule ----------------
            q_proj()
            aq_cm.__exit__(None, None, None)
            w1s_cm = tc.tile_pool(name="p_w1s", bufs=4, side="right")
            p_w1s = w1s_cm.__enter__()

            h1g0 = h1_load(0)
            for u in k_units(0, h1g0):
                u()
            for u in v_units(range(0, 4), h1g0):
                u()
            h1g1 = h1_load(1)
            attn(0, fillers=k_units(1, h1g1) + v_units(range(4, 8), h1g1))
            h1g2 = h1_load(2)
            attn(1, fillers=k_units(2, h1g2) + v_units(range(8, 12), h1g2))
            h1g3 = h1_load(3)
            attn(2, fillers=c_units(0) + [lambda: phase_D(0)]
                 + k_units(3, h1g3) + v_units(range(12, 16), h1g3))
            h1s_cm.__exit__(None, None, None)
            a_cm.__exit__(None, None, None)
            g1_cm = tc.tile_pool(name="p_g1", bufs=1, side="right")
            p_g1 = g1_cm.__enter__()
            g1g0 = p_g1.tile([P, NCF, 512], DT.bfloat16, tag="g1")
            attn(3, fillers=e_stage_units(g1g0, 0))
            last_exp3 = attn.last_exp
            phase_C(1)
            phase_D(1)

            qkv_cm.__exit__(None, None, None)
            w2s_cm = tc.tile_pool(name="p_w2s", bufs=3, side="left")
            p_w2s = w2s_cm.__enter__()

            e_gelu_batch(g1g0, last_exp3)
            phase_F(0, g1g0, p_w2s)
            g1g1 = p_g1.tile([P, NCF, 512], DT.bfloat16, tag="g1")
            phase_E(1, g1g1)
            phase_F(1, g1g1, p_w2s)

            w2s_cm.__exit__(None, None, None)
            g1_cm.__exit__(None, None, None)
            w1s_cm.__exit__(None, None, None)
            rep_cm.close()

    nc.compile()
    return nc


# ---------------- host side ----------------

def host_prep(x, Wq, Wk, Wv, Wo, bo, W1, b1, W2, b2, g1, be1, g2, be2):
    D = x.shape[2]
    H = Wq.shape[0]
    FF = W1.shape[1]
    NCF = FF // P
    f32 = np.float32

    mu = x.mean(-1, keepdims=True)
    var = ((x - mu) ** 2).mean(-1, keepdims=True)
    h1 = ((x - mu) / np.sqrt(var + EPS) * g1 + be1).astype(f32)

    shared = dict(
        wq=np.ascontiguousarray(Wq.transpose(1, 0, 2).reshape(D, -1)).astype(BF),
        wk=np.ascontiguousarray(Wk.transpose(1, 0, 2).reshape(D, -1)).astype(BF),
        wv=np.ascontiguousarray(Wv.transpose(1, 0, 2).reshape(D, -1)).astype(BF),
        wo=np.ascontiguousarray(Wo).astype(BF),
        w1=np.ascontiguousarray(g2[:, None] * W1).astype(BF),
        w2=np.ascontiguousarray(W2).astype(BF),
        b1c=np.ascontiguousarray((b1 + be2 @ W1).astype(f32).reshape(NCF, P).T),
        ident=np.eye(P, dtype=BF),
    )
    per_core = []
    for c in range(8):
        b, p = c // 2, c % 2
        r = np.arange(P)[:, None]
        j = np.arange(256)[None, :]
        qoff = np.where(j < P, 2 * j + p, 256 + 2 * (j - P) + p)
        m4 = np.zeros((P, 4, 256), f32)
        for t in range(4):
            m4[:, t, :] = (P * t + r <= qoff)
        m = np.concatenate([m4[:, 0::2, :], m4[:, 1::2, :]], axis=2)  # [P,2,512]
        per_core.append(dict(
            h1t=np.ascontiguousarray(h1[b].T).astype(BF),
            h1qt=np.ascontiguousarray(h1[b, p::2, :].T).astype(BF),
            xqt=np.ascontiguousarray(
                x[b, p::2, :].T + bo[:, None] + b2[:, None]).astype(f32),
            maskc=m.astype(BF),
            **shared,
        ))
    return per_core


def assemble(outts, B, T, D):
    out = np.zeros((B, T, D), np.float32)
    for c in range(8):
        b, p = c // 2, c % 2
        out[b, p::2, :] = outts[c].T
    return out


# ======================== top-level kernel entry ========================

_CACHE = {}


def _get_program():
    if "nc" not in _CACHE:
        _CACHE["nc"] = build_program(1024, 2048, 16, 64, 4096)
    return _CACHE["nc"]


def kernel(**inputs):
    """Full transformer block on 8 TRN2 NeuronCores.

    Takes the full unsharded inputs (as produced by setup_inputs()), shards
    (batch x query-parity) across 8 cores, runs the Bass SPMD kernel, and
    reassembles the full [4, 2048, 1024] float32 output.
    """
    from concourse.bass_utils import run_bass_kernel_spmd

    np_inputs = {k: np.asarray(v, np.float32) for k, v in inputs.items()}
    B, T, D = np_inputs["x"].shape
    nc = _get_program()
    per_core = host_prep(**np_inputs)
    res = run_bass_kernel_spmd(nc, per_core, list(range(8)))
    outts = [res.results[c]["outt"] for c in range(8)]
    return assemble(outts, B, T, D)


# revision 52
# speedup vs baseline: 1.0267x; 1.0267x over previous
"""Parametrized Bass/Tile transformer-block kernel for TRN2, 8-core SPMD.

Sharding: core c -> (batch b=c//2, query parity p=c%2). Each core computes the
output rows for tokens {2t+p} of its batch element. Fully uniform program, no
collectives; causal structure is uniform because local q-block i attends
exactly 2i+2 key blocks on every core (parity handled by a fixed mask).

v2 layout: single fused dataflow region. Q projections first (frees h1qt for
the streamed-W1 buffer), K/V projections interleaved between attention
superblocks so the PE stays fed while ACT chews softmax exps, Wo+LN2 per
512-token group as soon as its attention superblocks land, FFN streamed
per-group afterwards (W1/W2 chunks DMA'd just-in-time into space freed by
stage-A pools). Attention operands (q/k/v) are fp8e4m3 in SBUF; exps are
computed on [P,1024] PSUM spans (2 banks) to halve ACT instruction count.
LN2's rstd uses exp(-0.5*ln(var+eps)) so the whole pre-FFN phase stays on the
natural_log_exp ACT table set; gelus are dep-ordered after the last exp so
there is exactly one table switch.
"""
import sys
for _p in ("/opt/trn_rl_repo", "/root/.axon_site/_ro/trn_rl_repo"):
    if _p not in sys.path:
        sys.path.append(_p)

from contextlib import ExitStack

import numpy as np
import ml_dtypes

import concourse.bass as bass
import concourse.mybir as mybir
import concourse.tile as tile
from concourse import bacc
from concourse.bass import ts, ds
from concourse.tile_rust import add_dep_helper

AF = mybir.ActivationFunctionType
DT = mybir.dt
BF = ml_dtypes.bfloat16
P = 128
EPS = 1e-5


def build_program(D, T, H, DK, FF, reps=1):
    Tq = T // 2
    NCD = D // P            # feature chunks
    NCF = FF // P           # hidden chunks
    NHP = H // 2            # head pairs
    NQG = Tq // 512         # 512-token groups over own queries
    NM = Tq // 256          # attention q superblocks (256 wide)
    NKB = T // P            # key blocks
    WV = min(512, D)        # V-projection column group width
    NVG = D // WV
    HPG = WV // DK          # heads per V col group
    F8 = DT.float8e4
    assert D == H * DK and Tq % 512 == 0 and NHP == NCD

    nc = bacc.Bacc(None, target_bir_lowering=False)

    # ---- DRAM I/O ----
    h1t_d = nc.dram_tensor("h1t", [D, T], DT.bfloat16, kind="ExternalInput")
    h1qt_d = nc.dram_tensor("h1qt", [D, Tq], DT.bfloat16, kind="ExternalInput")
    xqt_d = nc.dram_tensor("xqt", [D, Tq], DT.float32, kind="ExternalInput")
    wq_d = nc.dram_tensor("wq", [D, D], DT.bfloat16, kind="ExternalInput")
    wk_d = nc.dram_tensor("wk", [D, D], DT.bfloat16, kind="ExternalInput")
    wv_d = nc.dram_tensor("wv", [D, D], DT.bfloat16, kind="ExternalInput")
    wo_d = nc.dram_tensor("wo", [D, D], DT.bfloat16, kind="ExternalInput")
    w1_d = nc.dram_tensor("w1", [D, FF], DT.bfloat16, kind="ExternalInput")
    w2_d = nc.dram_tensor("w2", [FF, D], DT.bfloat16, kind="ExternalInput")
    b1c_d = nc.dram_tensor("b1c", [P, NCF], DT.float32, kind="ExternalInput")
    mask_d = nc.dram_tensor("maskc", [P, 2, 512], DT.bfloat16, kind="ExternalInput")
    ident_d = nc.dram_tensor("ident", [P, P], DT.bfloat16, kind="ExternalInput")
    outt_d = nc.dram_tensor("outt", [D, Tq], DT.float32, kind="ExternalOutput")

    h1t_r = h1t_d[:].rearrange("(c p) t -> p c t", p=P)
    h1qt_r = h1qt_d[:].rearrange("(c p) t -> p c t", p=P)
    xqt_r = xqt_d[:].rearrange("(c p) t -> p c t", p=P)
    wq_r = wq_d[:].rearrange("(c p) n -> p c n", p=P)
    wk_r = wk_d[:].rearrange("(c p) n -> p c n", p=P)
    wv_r = wv_d[:].rearrange("(c p) n -> p c n", p=P)
    wo_r = wo_d[:].rearrange("(c p) n -> p c n", p=P)
    w1_r = w1_d[:].rearrange("(c p) n -> p c n", p=P)
    w2_r = w2_d[:].rearrange("(c p) n -> p c n", p=P)
    mask_flat = mask_d[:].rearrange("p a b -> p (a b)")
    outt_r = outt_d[:].rearrange("(c p) t -> p c t", p=P)

    with tile.TileContext(nc) as tc, ExitStack() as top:
        constp = top.enter_context(tc.tile_pool(name="const", bufs=1))
        mask_sb = constp.tile([P, 1024], DT.bfloat16)
        nc.scalar.dma_start(mask_sb[:], mask_flat)
        ident_sb = constp.tile([P, P], DT.bfloat16)
        nc.scalar.dma_start(ident_sb[:], ident_d[:])
        # additive causal mask: 0 where allowed, -30000 where masked
        maskm = constp.tile([P, 1024], DT.bfloat16)
        nc.scalar.activation(maskm[:], mask_sb[:], AF.Copy,
                             bias=-30000.0, scale=30000.0)
        b1c_sb = constp.tile([P, NCF], DT.float32)
        nc.scalar.dma_start(b1c_sb[:], b1c_d[:])
        ones_bf = constp.tile([P, 1], DT.bfloat16)
        nc.vector.memset(ones_bf[:], 1.0)
        ones_f = constp.tile([1, P], DT.float32)
        nc.vector.memset(ones_f[:], 1.0)
        ones_r = constp.tile([1, P], DT.float32r)
        with nc.allow_low_precision(reason="f32r ones"):
            nc.vector.tensor_copy(ones_r[:], ones_f[:])
        zero_b = constp.tile([P, 1], DT.float32)
        nc.vector.memset(zero_b[:], 0.0)
        zero1 = constp.tile([1, 1], DT.float32)
        nc.vector.memset(zero1[:], 0.0)
        eps_b = constp.tile([1, 1], DT.float32)
        nc.vector.memset(eps_b[:], EPS)

        for _rep in range(reps):
            # ---------------- pools ----------------
            qkv_cm = tc.tile_pool(name="p_qkv", bufs=1, side="left")
            p_qkv = qkv_cm.__enter__()
            qt_sb = p_qkv.tile([P, NHP, Tq], F8)
            kt_sb = p_qkv.tile([P, NHP, T], F8)
            v_sb = p_qkv.tile([P, NKB, 65 * H], F8)
            a_cm = tc.tile_pool(name="p_a", bufs=1, side="left")
            p_a = a_cm.__enter__()
            wv_sb = p_a.tile([P, NCD, D], DT.bfloat16)
            h1s_cm = tc.tile_pool(name="p_h1s", bufs=2, side="left")
            p_h1s = h1s_cm.__enter__()
            aq_cm = tc.tile_pool(name="p_aq", bufs=1, side="left")
            p_aq = aq_cm.__enter__()
            h1qt_sb = p_aq.tile([P, NCD, Tq], DT.bfloat16)

            rep_cm = ExitStack()
            w_a = rep_cm.enter_context(
                tc.tile_pool(name="w_a", bufs=3, side="right"))
            p_wor = rep_cm.enter_context(
                tc.tile_pool(name="p_wor", bufs=1, side="right"))
            wo_sb = p_wor.tile([P, NCD, D], DT.bfloat16)
            p_oxx = rep_cm.enter_context(
                tc.tile_pool(name="p_oxx", bufs=1, side="right"))
            ot_sb = p_oxx.tile([P, NCD, Tq], DT.bfloat16)
            x1_sb = p_oxx.tile([P, NCD, Tq], DT.bfloat16)
            xh_sb = p_oxx.tile([P, NCD, Tq], DT.bfloat16)
            p_exp = rep_cm.enter_context(
                tc.tile_pool(name="p_exp", bufs=3, side="right"))
            p_nrm = rep_cm.enter_context(
                tc.tile_pool(name="p_nrm", bufs=1, side="right"))
            p_sq = rep_cm.enter_context(
                tc.tile_pool(name="p_sq", bufs=1, side="right"))
            p_st = rep_cm.enter_context(
                tc.tile_pool(name="p_st", bufs=1, side="right"))
            p_xq = rep_cm.enter_context(
                tc.tile_pool(name="p_xq", bufs=2, side="right"))
            p_out = rep_cm.enter_context(
                tc.tile_pool(name="p_out", bufs=2, side="right"))
            ps_s = rep_cm.enter_context(
                tc.tile_pool(name="ps_s", bufs=2, space="PSUM"))
            ps_av = rep_cm.enter_context(
                tc.tile_pool(name="ps_av", bufs=2, space="PSUM"))
            ps_main = rep_cm.enter_context(
                tc.tile_pool(name="ps_m", bufs=2, space="PSUM"))

            # ---------------- startup DMAs ----------------
            for c in range(NCD):
                nc.gpsimd.dma_start(
                    h1qt_sb[:, c, ds(0, 512)], h1qt_r[:, c, ds(0, 512)])
            for c in range(NCD):
                nc.gpsimd.dma_start(
                    h1qt_sb[:, c, ds(512, 512)], h1qt_r[:, c, ds(512, 512)])
            for c in range(NCD):
                nc.scalar.dma_start(wv_sb[:, c], wv_r[:, c])
            for c in range(NCD):
                nc.scalar.dma_start(wo_sb[:, c], wo_r[:, c])
            for h in range(H):
                nc.vector.memset(v_sb[:, :, ds(65 * h + 64, 1)], 1.0)

            def h1_load(g):
                h1g = p_h1s.tile([P, NCD, 512], DT.bfloat16, tag="h1g")
                for c in range(NCD):
                    nc.gpsimd.dma_start(
                        h1g[:, c], h1t_r[:, c, ds(512 * g, 512)])
                return h1g

            # ---------------- stage A pieces ----------------
            def q_proj():
                for g in range(NQG):
                    for hp in range(NHP):
                        wq_t = w_a.tile([P, NCD, P], DT.bfloat16, tag="wqk")
                        nc.sync.dma_start(wq_t[:], wq_r[:, :, ds(P * hp, P)])
                        ps_q = ps_main.tile([P, 512], DT.float32, tag="ps")
                        for c in range(NCD):
                            nc.tensor.matmul(
                                ps_q[:], wq_t[:, c],
                                h1qt_sb[:, c, ds(512 * g, 512)],
                                start=(c == 0), stop=(c == NCD - 1))
                        with nc.allow_low_precision(reason="fp8 attention operand"):
                            nc.vector.tensor_copy(
                                qt_sb[:, hp, ds(512 * g, 512)], ps_q[:])

            def k_unit(g, h1g, hp):
                wk_t = w_a.tile([P, NCD, P], DT.bfloat16, tag="wqk")
                nc.sync.dma_start(wk_t[:], wk_r[:, :, ds(P * hp, P)])
                ps_k = ps_main.tile([P, 512], DT.float32, tag="ps")
                for c in range(NCD):
                    nc.tensor.matmul(
                        ps_k[:], wk_t[:, c], h1g[:, c],
                        start=(c == 0), stop=(c == NCD - 1))
                with nc.allow_low_precision(reason="fp8 attention operand"):
                    nc.vector.tensor_copy(kt_sb[:, hp, ds(512 * g, 512)], ps_k[:])

            def v_unit(tb, g2, h1g):
                ps_v = ps_main.tile([P, WV], DT.float32, tag="ps")
                for c in range(NCD):
                    nc.tensor.matmul(
                        ps_v[:], h1g[:, c, ds(P * (tb % 4), P)],
                        wv_sb[:, c, ds(WV * g2, WV)],
                        start=(c == 0), stop=(c == NCD - 1))
                with nc.allow_low_precision(reason="fp8 attention operand"):
                    for hh in range(HPG):
                        h = HPG * g2 + hh
                        nc.vector.tensor_copy(
                            v_sb[:, tb, ds(65 * h, 64)],
                            ps_v[:, ds(64 * hh, 64)])

            def k_units(g, h1g):
                return [lambda hp=hp: k_unit(g, h1g, hp) for hp in range(NHP)]

            def v_units(tbs, h1g):
                return [lambda tb=tb, g2=g2: v_unit(tb, g2, h1g)
                        for tb in tbs for g2 in range(NVG)]

            # ---------------- attention superblock ----------------
            def attn_norm(mp, hp, po):
                rec = p_nrm.tile([1, 512], DT.float32r, tag="rec")
                with nc.allow_low_precision(reason="f32r broadcast operand"):
                    nc.vector.reciprocal(rec[:], po[64:65, :])
                pbc = ps_main.tile([P, 512], DT.float32, tag="ps", name="pbc")
                nc.tensor.matmul(pbc[0:64, :], ones_r[:, 0:64], rec[:],
                                 start=True, stop=True)
                rb = p_nrm.tile([64, 512], DT.float32, tag="rb")
                nc.vector.tensor_copy(rb[:], pbc[0:64, :])
                for hi in range(2):
                    nc.vector.tensor_mul(
                        ot_sb[64 * hi:64 * hi + 64, hp, ds(256 * mp, 256)],
                        po[0:64, ds(256 * hi, 256)], rb[:, ds(256 * hi, 256)])

            def attn(mp, fillers=()):
                nkv = 4 * mp + 4
                ngrp = nkv // 4
                scale = float(DK) ** -0.5
                L = len(fillers)
                pending = None
                for hp in range(NHP):
                    po = ps_av.tile([P, 512], DT.float32, tag="po")
                    for gk in range(ngrp):
                        band = gk == ngrp - 1
                        pss = [ps_s.tile([P, 1024], DT.float32, tag="ps_s",
                                         name=f"pss{_i}") for _i in range(2)]
                        for j in range(4):
                            kb = 4 * gk + j
                            for hi in range(2):
                                pb = 64 * hi
                                nc.tensor.matmul(
                                    pss[hi][:, ds(256 * j, 256)],
                                    kt_sb[pb:pb + 64, hp, ds(P * kb, P)],
                                    qt_sb[pb:pb + 64, hp, ds(256 * mp, 256)],
                                    start=(j % 2 == 0),
                                    stop=(j % 2 == 1) and not band,
                                    tile_position=(pb, 0),
                                    skip_group_check=True)
                        if band:
                            # fold the causal mask in additively on the PE:
                            # exp(-30000/8) underflows to exactly 0
                            for hi in range(2):
                                for s in range(2):
                                    nc.tensor.matmul(
                                        pss[hi][:, ds(512 * s, 512)],
                                        ident_sb[:],
                                        maskm[:, ds(512 * s, 512)],
                                        start=False, stop=True,
                                        skip_group_check=True)
                        exs = []
                        for hi in range(2):
                            ex = p_exp.tile([P, 2, 2, 256], F8, tag="ex")
                            with nc.allow_low_precision(reason="fp8 softmax wts"):
                                a = nc.scalar.activation(ex[:], pss[hi][:], AF.Exp,
                                                         bias=zero_b[:], scale=scale)
                            attn.last_exp = a.ins if hasattr(a, "ins") else a
                            exs.append(ex)
                        for pp in range(2):
                            for hi in range(2):
                                h = 2 * hp + hi
                                nc.tensor.matmul(
                                    po[0:65, ds(256 * hi, 256)],
                                    v_sb[:, ds(4 * gk + 2 * pp, 2), ds(65 * h, 65)],
                                    exs[hi][:, pp],
                                    start=(gk == 0 and pp == 0 and hi == 0),
                                    stop=(gk == ngrp - 1 and pp == 1 and hi == 1),
                                    perf_mode=mybir.MatmulPerfMode.DoubleRow,
                                    skip_group_check=True)
                    # norm chain runs one hp behind so it never heads the PE queue
                    if pending is not None:
                        attn_norm(mp, pending[0], pending[1])
                    pending = (hp, po)
                    for u in fillers[L * hp // NHP: L * (hp + 1) // NHP]:
                        u()
                attn_norm(mp, pending[0], pending[1])

            # ---------------- Wo + residual per group ----------------
            def c_unit(g, c):
                ps_x = ps_main.tile([P, 512], DT.float32, tag="ps")
                for k in range(NCD):
                    nc.tensor.matmul(
                        ps_x[:], wo_sb[:, k, ds(P * c, P)],
                        ot_sb[:, k, ds(512 * g, 512)],
                        start=(k == 0), stop=(k == NCD - 1))
                xq_t = p_xq.tile([P, 512], DT.float32, tag="xq")
                nc.sync.dma_start(xq_t[:], xqt_r[:, c, ds(512 * g, 512)])
                nc.vector.tensor_add(
                    x1_sb[:, c, ds(512 * g, 512)], ps_x[:], xq_t[:])

            def phase_C(g):
                for c in range(NCD):
                    c_unit(g, c)

            def c_units(g):
                return [lambda c=c: c_unit(g, c) for c in range(NCD)]

            # ---------------- LN2 per group ----------------
            def phase_D(g):
                sl = ds(512 * g, 512)
                ps1 = ps_main.tile([P, 512], DT.float32, tag="ps", name="ps1")
                for c in range(NCD):
                    nc.tensor.matmul(ps1[0:1, :], ones_bf[:], x1_sb[:, c, sl],
                                     start=(c == 0), stop=(c == NCD - 1))
                ps2 = ps_main.tile([P, 512], DT.float32, tag="ps", name="ps2")
                for c in range(NCD):
                    sq = p_sq.tile([P, 512], DT.bfloat16, tag="sq")
                    nc.scalar.activation(sq[:], x1_sb[:, c, sl], AF.Square,
                                         bias=zero_b[:])
                    nc.tensor.matmul(ps2[0:1, :], ones_bf[:], sq[:],
                                     start=(c == 0), stop=(c == NCD - 1))
                mu = p_st.tile([1, 512], DT.float32, tag="mu")
                nc.vector.tensor_scalar_mul(mu[:], ps1[0:1, :], 1.0 / D)
                msq = p_st.tile([1, 512], DT.float32, tag="msq")
                nc.vector.tensor_mul(msq[:], mu[:], mu[:])
                nc.vector.scalar_tensor_tensor(
                    msq[:], ps2[0:1, :], 1.0 / D, msq[:],
                    mybir.AluOpType.mult, mybir.AluOpType.subtract)
                # rstd = exp(-0.5 * ln(var + eps)) — stays on the exp/ln table set
                lnv = p_st.tile([1, 512], DT.float32, tag="lnv")
                nc.scalar.activation(lnv[:], msq[:], AF.Ln, bias=eps_b[:])
                r2f = p_st.tile([1, 512], DT.float32, tag="r2f")
                last_act = nc.scalar.activation(r2f[:], lnv[:], AF.Exp,
                                                bias=zero1[:], scale=-0.5)
                r2 = p_st.tile([1, 512], DT.float32r, tag="r2")
                with nc.allow_low_precision(reason="f32r broadcast operand"):
                    nc.vector.tensor_copy(r2[:], r2f[:])
                m2r = p_st.tile([1, 512], DT.float32r, tag="m2r")
                with nc.allow_low_precision(reason="f32r broadcast operand"):
                    nc.vector.tensor_mul(m2r[:], mu[:], r2[:])
                pb1 = ps_main.tile([P, 512], DT.float32, tag="ps", name="pb1")
                nc.tensor.matmul(pb1[:], ones_r[:], r2[:], start=True, stop=True)
                r2b = p_sq.tile([P, 512], DT.float32, tag="r2b")
                nc.vector.tensor_copy(r2b[:], pb1[:])
                pb2 = ps_main.tile([P, 512], DT.float32, tag="ps", name="pb2")
                nc.tensor.matmul(pb2[:], ones_r[:], m2r[:], start=True, stop=True)
                m2rb = p_sq.tile([P, 512], DT.float32, tag="m2rb")
                nc.vector.tensor_copy(m2rb[:], pb2[:])
                for c in range(NCD):
                    tmp = p_sq.tile([P, 512], DT.float32, tag="tmp")
                    nc.vector.tensor_mul(tmp[:], x1_sb[:, c, sl], r2b[:])
                    nc.vector.tensor_sub(xh_sb[:, c, sl], tmp[:], m2rb[:])
                return last_act.ins if hasattr(last_act, "ins") else last_act

            # ---------------- FFN per group ----------------
            def e_mm_chain(g1g, m, g):
                w1_t = p_w1s.tile([P, NCD, P], DT.bfloat16, tag="w1")
                nc.scalar.dma_start(w1_t[:], w1_r[:, :, ds(P * m, P)])
                ps_f = ps_main.tile([P, 512], DT.float32, tag="ps")
                for c in range(NCD):
                    nc.tensor.matmul(
                        ps_f[:], w1_t[:, c],
                        xh_sb[:, c, ds(512 * g, 512)],
                        start=(c == 0), stop=(c == NCD - 1))
                return ps_f

            def e_stage_units(g1g, g):
                # matmul + bf16 staging copy only; gelu deferred so these can
                # interleave with attention without ACT table thrash
                def unit(m):
                    ps_f = e_mm_chain(g1g, m, g)
                    nc.vector.tensor_copy(g1g[:, m], ps_f[:])
                return [lambda m=m: unit(m) for m in range(NCF)]

            def e_gelu_batch(g1g, dep_inst=None):
                for m in range(NCF):
                    a = nc.scalar.activation(g1g[:, m], g1g[:, m], AF.Gelu,
                                             bias=b1c_sb[:, ds(m, 1)])
                    if dep_inst is not None:
                        # every gelu must wait: the scheduler slots ACT ops by
                        # availability, and a gelu between exps costs two
                        # ~1.3us ACT-table loads
                        add_dep_helper(
                            a.ins if hasattr(a, "ins") else a, dep_inst,
                            sync=False, reason="gelu batch after attn exps")

            def phase_E(g, g1g):
                for m in range(NCF):
                    ps_f = e_mm_chain(g1g, m, g)
                    nc.scalar.activation(g1g[:, m], ps_f[:], AF.Gelu,
                                         bias=b1c_sb[:, ds(m, 1)])

            def phase_F(g, g1g, p_w2s):
                for c in range(NCD):
                    w2_t = p_w2s.tile([P, NCF, P], DT.bfloat16, tag="w2")
                    nc.sync.dma_start(w2_t[:], w2_r[:, :, ds(P * c, P)])
                    pf = ps_main.tile([P, 512], DT.float32, tag="ps")
                    for hh in range(NCF):
                        nc.tensor.matmul(pf[:], w2_t[:, hh], g1g[:, hh],
                                         start=(hh == 0), stop=(hh == NCF - 1))
                    out_t = p_out.tile([P, 512], DT.float32, tag="out")
                    nc.vector.tensor_add(out_t[:], pf[:],
                                         x1_sb[:, c, ds(512 * g, 512)])
                    nc.sync.dma_start(outt_r[:, c, ds(512 * g, 512)], out_t[:])

            # ---------------- emission schedule ----------------
            q_proj()
            aq_cm.__exit__(None, None, None)
            w1s_cm = tc.tile_pool(name="p_w1s", bufs=4, side="right")
            p_w1s = w1s_cm.__enter__()

            h1g0 = h1_load(0)
            for u in k_units(0, h1g0):
                u()
            for u in v_units(range(0, 4), h1g0):
                u()
            h1g1 = h1_load(1)
            attn(0, fillers=k_units(1, h1g1) + v_units(range(4, 8), h1g1))
            h1g2 = h1_load(2)
            attn(1, fillers=k_units(2, h1g2) + v_units(range(8, 12), h1g2))
            h1g3 = h1_load(3)
            attn(2, fillers=c_units(0) + [lambda: phase_D(0)]
                 + k_units(3, h1g3) + v_units(range(12, 16), h1g3))
            h1s_cm.__exit__(None, None, None)
            a_cm.__exit__(None, None, None)
            g1_cm = tc.tile_pool(name="p_g1", bufs=1, side="right")
            p_g1 = g1_cm.__enter__()
            g1g0 = p_g1.tile([P, NCF, 512], DT.bfloat16, tag="g1")
            attn(3, fillers=e_stage_units(g1g0, 0))
            last_exp3 = attn.last_exp
            phase_C(1)
            phase_D(1)

            qkv_cm.__exit__(None, None, None)
            w2s_cm = tc.tile_pool(name="p_w2s", bufs=3, side="left")
            p_w2s = w2s_cm.__enter__()

            e_gelu_batch(g1g0, last_exp3)
            phase_F(0, g1g0, p_w2s)
            g1g1 = p_g1.tile([P, NCF, 512], DT.bfloat16, tag="g1")
            phase_E(1, g1g1)
            phase_F(1, g1g1, p_w2s)

            w2s_cm.__exit__(None, None, None)
            g1_cm.__exit__(None, None, None)
            w1s_cm.__exit__(None, None, None)
            rep_cm.close()

    nc.compile()
    return nc


# ---------------- host side ----------------

def host_prep(x, Wq, Wk, Wv, Wo, bo, W1, b1, W2, b2, g1, be1, g2, be2):
    D = x.shape[2]
    H = Wq.shape[0]
    FF = W1.shape[1]
    NCF = FF // P
    f32 = np.float32

    mu = x.mean(-1, keepdims=True)
    var = ((x - mu) ** 2).mean(-1, keepdims=True)
    h1 = ((x - mu) / np.sqrt(var + EPS) * g1 + be1).astype(f32)

    shared = dict(
        wq=np.ascontiguousarray(Wq.transpose(1, 0, 2).reshape(D, -1)).astype(BF),
        wk=np.ascontiguousarray(Wk.transpose(1, 0, 2).reshape(D, -1)).astype(BF),
        wv=np.ascontiguousarray(Wv.transpose(1, 0, 2).reshape(D, -1)).astype(BF),
        wo=np.ascontiguousarray(Wo).astype(BF),
        w1=np.ascontiguousarray(g2[:, None] * W1).astype(BF),
        w2=np.ascontiguousarray(W2).astype(BF),
        b1c=np.ascontiguousarray((b1 + be2 @ W1).astype(f32).reshape(NCF, P).T),
        ident=np.eye(P, dtype=BF),
    )
    per_core = []
    for c in range(8):
        b, p = c // 2, c % 2
        r = np.arange(P)[:, None]
        j = np.arange(256)[None, :]
        qoff = np.where(j < P, 2 * j + p, 256 + 2 * (j - P) + p)
        m4 = np.zeros((P, 4, 256), f32)
        for t in range(4):
            m4[:, t, :] = (P * t + r <= qoff)
        m = np.concatenate([m4[:, 0::2, :], m4[:, 1::2, :]], axis=2)  # [P,2,512]
        per_core.append(dict(
            h1t=np.ascontiguousarray(h1[b].T).astype(BF),
            h1qt=np.ascontiguousarray(h1[b, p::2, :].T).astype(BF),
            xqt=np.ascontiguousarray(
                x[b, p::2, :].T + bo[:, None] + b2[:, None]).astype(f32),
            maskc=m.astype(BF),
            **shared,
        ))
    return per_core


def assemble(outts, B, T, D):
    out = np.zeros((B, T, D), np.float32)
    for c in range(8):
        b, p = c // 2, c % 2
        out[b, p::2, :] = outts[c].T
    return out


# ======================== top-level kernel entry ========================

_CACHE = {}


def _get_program():
    if "nc" not in _CACHE:
        _CACHE["nc"] = build_program(1024, 2048, 16, 64, 4096)
    return _CACHE["nc"]


def kernel(**inputs):
    """Full transformer block on 8 TRN2 NeuronCores.

    Takes the full unsharded inputs (as produced by setup_inputs()), shards
    (batch x query-parity) across 8 cores, runs the Bass SPMD kernel, and
    reassembles the full [4, 2048, 1024] float32 output.
    """
    from concourse.bass_utils import run_bass_kernel_spmd

    np_inputs = {k: np.asarray(v, np.float32) for k, v in inputs.items()}
    B, T, D = np_inputs["x"].shape
    nc = _get_program()
    per_core = host_prep(**np_inputs)
    res = run_bass_kernel_spmd(nc, per_core, list(range(8)))
    outts = [res.results[c]["outt"] for c in range(8)]
    return assemble(outts, B, T, D)


# revision 66
# speedup vs baseline: 1.0570x; 1.0295x over previous
"""Parametrized Bass/Tile transformer-block kernel for TRN2, 8-core SPMD.

Sharding: core c -> (batch b=c//2, query parity p=c%2). Each core computes the
output rows for tokens {2t+p} of its batch element. Fully uniform program, no
collectives; causal structure is uniform because local q-block i attends
exactly 2i+2 key blocks on every core (parity handled by a fixed mask).

v2 layout: single fused dataflow region. Q projections first (frees h1qt for
the streamed-W1 buffer), K/V projections interleaved between attention
superblocks so the PE stays fed while ACT chews softmax exps, Wo+LN2 per
512-token group as soon as its attention superblocks land, FFN streamed
per-group afterwards (W1/W2 chunks DMA'd just-in-time into space freed by
stage-A pools). Attention operands (q/k/v) are fp8e4m3 in SBUF; exps are
computed on [P,1024] PSUM spans (2 banks) to halve ACT instruction count.
LN2's rstd uses exp(-0.5*ln(var+eps)) so the whole pre-FFN phase stays on the
natural_log_exp ACT table set; gelus are dep-ordered after the last exp so
there is exactly one table switch.
"""
import sys
for _p in ("/opt/trn_rl_repo", "/root/.axon_site/_ro/trn_rl_repo"):
    if _p not in sys.path:
        sys.path.append(_p)

from contextlib import ExitStack

import numpy as np
import ml_dtypes

import concourse.bass as bass
import concourse.mybir as mybir
import concourse.tile as tile
from concourse import bacc
from concourse.bass import ts, ds
from concourse.tile_rust import add_dep_helper

AF = mybir.ActivationFunctionType
DT = mybir.dt
BF = ml_dtypes.bfloat16
P = 128
EPS = 1e-5


def build_program(D, T, H, DK, FF, reps=1):
    Tq = T // 2
    NCD = D // P            # feature chunks
    NCF = FF // P           # hidden chunks
    NHP = H // 2            # head pairs
    NQG = Tq // 512         # 512-token groups over own queries
    NM = Tq // 256          # attention q superblocks (256 wide)
    NKB = T // P            # key blocks
    WV = min(512, D)        # V-projection column group width
    NVG = D // WV
    HPG = WV // DK          # heads per V col group
    F8 = DT.float8e4
    assert D == H * DK and Tq % 512 == 0 and NHP == NCD

    nc = bacc.Bacc(None, target_bir_lowering=False)

    # ---- DRAM I/O ----
    h1t_d = nc.dram_tensor("h1t", [D, T], DT.bfloat16, kind="ExternalInput")
    h1qt_d = nc.dram_tensor("h1qt", [D, Tq], DT.bfloat16, kind="ExternalInput")
    xqt_d = nc.dram_tensor("xqt", [D, Tq], DT.float32, kind="ExternalInput")
    wq_d = nc.dram_tensor("wq", [D, D], DT.bfloat16, kind="ExternalInput")
    wk_d = nc.dram_tensor("wk", [D, D], DT.bfloat16, kind="ExternalInput")
    wv_d = nc.dram_tensor("wv", [D, D], DT.bfloat16, kind="ExternalInput")
    wo_d = nc.dram_tensor("wo", [D, D], DT.bfloat16, kind="ExternalInput")
    w1_d = nc.dram_tensor("w1", [D, FF], DT.bfloat16, kind="ExternalInput")
    w2_d = nc.dram_tensor("w2", [FF, D], DT.bfloat16, kind="ExternalInput")
    b1c_d = nc.dram_tensor("b1c", [P, NCF], DT.float32, kind="ExternalInput")
    mask_d = nc.dram_tensor("maskc", [P, 2, 512], DT.bfloat16, kind="ExternalInput")
    ident_d = nc.dram_tensor("ident", [P, P], DT.bfloat16, kind="ExternalInput")
    outt_d = nc.dram_tensor("outt", [D, Tq], DT.float32, kind="ExternalOutput")

    h1t_r = h1t_d[:].rearrange("(c p) t -> p c t", p=P)
    h1qt_r = h1qt_d[:].rearrange("(c p) t -> p c t", p=P)
    xqt_r = xqt_d[:].rearrange("(c p) t -> p c t", p=P)
    wq_r = wq_d[:].rearrange("(c p) n -> p c n", p=P)
    wk_r = wk_d[:].rearrange("(c p) n -> p c n", p=P)
    wv_r = wv_d[:].rearrange("(c p) n -> p c n", p=P)
    wo_r = wo_d[:].rearrange("(c p) n -> p c n", p=P)
    w1_r = w1_d[:].rearrange("(c p) n -> p c n", p=P)
    w2_r = w2_d[:].rearrange("(c p) n -> p c n", p=P)
    mask_flat = mask_d[:].rearrange("p a b -> p (a b)")
    outt_r = outt_d[:].rearrange("(c p) t -> p c t", p=P)

    with tile.TileContext(nc) as tc, ExitStack() as top:
        constp = top.enter_context(tc.tile_pool(name="const", bufs=1))
        mask_sb = constp.tile([P, 1024], DT.bfloat16)
        nc.scalar.dma_start(mask_sb[:], mask_flat)
        ident_sb = constp.tile([P, P], DT.bfloat16)
        nc.scalar.dma_start(ident_sb[:], ident_d[:])
        # additive causal mask: 0 where allowed, -30000 where masked
        maskm = constp.tile([P, 1024], DT.bfloat16)
        nc.scalar.activation(maskm[:], mask_sb[:], AF.Copy,
                             bias=-30000.0, scale=30000.0)
        b1c_sb = constp.tile([P, NCF], DT.float32)
        nc.scalar.dma_start(b1c_sb[:], b1c_d[:])
        ones_bf = constp.tile([P, 1], DT.bfloat16)
        nc.vector.memset(ones_bf[:], 1.0)
        ones_f = constp.tile([1, P], DT.float32)
        nc.vector.memset(ones_f[:], 1.0)
        ones_r = constp.tile([1, P], DT.float32r)
        with nc.allow_low_precision(reason="f32r ones"):
            nc.vector.tensor_copy(ones_r[:], ones_f[:])
        zero_b = constp.tile([P, 1], DT.float32)
        nc.vector.memset(zero_b[:], 0.0)
        zero1 = constp.tile([1, 1], DT.float32)
        nc.vector.memset(zero1[:], 0.0)
        eps_b = constp.tile([1, 1], DT.float32)
        nc.vector.memset(eps_b[:], EPS)

        for _rep in range(reps):
            # ---------------- pools ----------------
            qkv_cm = tc.tile_pool(name="p_qkv", bufs=1, side="left")
            p_qkv = qkv_cm.__enter__()
            qt_sb = p_qkv.tile([P, NHP, Tq], F8)
            kt_sb = p_qkv.tile([P, NHP, T], F8)
            v_sb = p_qkv.tile([P, NKB, 65 * H], F8)
            a_cm = tc.tile_pool(name="p_a", bufs=1, side="left")
            p_a = a_cm.__enter__()
            wv_sb = p_a.tile([P, NCD, D], DT.bfloat16)
            h1s_cm = tc.tile_pool(name="p_h1s", bufs=2, side="left")
            p_h1s = h1s_cm.__enter__()
            aq_cm = tc.tile_pool(name="p_aq", bufs=1, side="left")
            p_aq = aq_cm.__enter__()
            h1qt_sb = p_aq.tile([P, NCD, Tq], DT.bfloat16)

            rep_cm = ExitStack()
            w_a = rep_cm.enter_context(
                tc.tile_pool(name="w_a", bufs=3, side="right"))
            p_wor = rep_cm.enter_context(
                tc.tile_pool(name="p_wor", bufs=1, side="right"))
            wo_sb = p_wor.tile([P, NCD, D], DT.bfloat16)
            p_oxx = rep_cm.enter_context(
                tc.tile_pool(name="p_oxx", bufs=1, side="right"))
            ot_sb = p_oxx.tile([P, NCD, Tq], DT.bfloat16)
            x1_sb = p_oxx.tile([P, NCD, Tq], DT.bfloat16)
            xh_sb = p_oxx.tile([P, NCD, Tq], DT.bfloat16)
            p_exp = rep_cm.enter_context(
                tc.tile_pool(name="p_exp", bufs=3, side="right"))
            p_nrm = rep_cm.enter_context(
                tc.tile_pool(name="p_nrm", bufs=1, side="right"))
            p_sq = rep_cm.enter_context(
                tc.tile_pool(name="p_sq", bufs=1, side="right"))
            p_st = rep_cm.enter_context(
                tc.tile_pool(name="p_st", bufs=1, side="right"))
            p_xq = rep_cm.enter_context(
                tc.tile_pool(name="p_xq", bufs=2, side="right"))
            p_out = rep_cm.enter_context(
                tc.tile_pool(name="p_out", bufs=2, side="right"))
            ps_s = rep_cm.enter_context(
                tc.tile_pool(name="ps_s", bufs=2, space="PSUM"))
            ps_av = rep_cm.enter_context(
                tc.tile_pool(name="ps_av", bufs=2, space="PSUM"))
            ps_main = rep_cm.enter_context(
                tc.tile_pool(name="ps_m", bufs=2, space="PSUM"))

            # ---------------- startup DMAs ----------------
            for c in range(NCD):
                nc.gpsimd.dma_start(
                    h1qt_sb[:, c, ds(0, 512)], h1qt_r[:, c, ds(0, 512)])
            for c in range(NCD):
                nc.gpsimd.dma_start(
                    h1qt_sb[:, c, ds(512, 512)], h1qt_r[:, c, ds(512, 512)])
            for c in range(NCD):
                nc.scalar.dma_start(wv_sb[:, c], wv_r[:, c])
            for c in range(NCD):
                nc.scalar.dma_start(wo_sb[:, c], wo_r[:, c])
            for h in range(H):
                nc.vector.memset(v_sb[:, :, ds(65 * h + 64, 1)], 1.0)

            def h1_load(g):
                h1g = p_h1s.tile([P, NCD, 512], DT.bfloat16, tag="h1g")
                for c in range(NCD):
                    nc.gpsimd.dma_start(
                        h1g[:, c], h1t_r[:, c, ds(512 * g, 512)])
                return h1g

            # ---------------- stage A pieces ----------------
            def q_proj():
                # hp-outer: each wq tile fetched once and used for both
                # 512-token groups — halves the sync-queue weight traffic so
                # k(0)'s wk fetches land sooner
                for hp in range(NHP):
                    wq_t = w_a.tile([P, NCD, P], DT.bfloat16, tag="wqk")
                    nc.sync.dma_start(wq_t[:], wq_r[:, :, ds(P * hp, P)])
                    for g in range(NQG):
                        ps_q = ps_main.tile([P, 512], DT.float32, tag="ps")
                        for c in range(NCD):
                            nc.tensor.matmul(
                                ps_q[:], wq_t[:, c],
                                h1qt_sb[:, c, ds(512 * g, 512)],
                                start=(c == 0), stop=(c == NCD - 1))
                        with nc.allow_low_precision(reason="fp8 attention operand"):
                            nc.vector.tensor_copy(
                                qt_sb[:, hp, ds(512 * g, 512)], ps_q[:])

            def k_unit(g, h1g, hp):
                wk_t = w_a.tile([P, NCD, P], DT.bfloat16, tag="wqk")
                nc.sync.dma_start(wk_t[:], wk_r[:, :, ds(P * hp, P)])
                ps_k = ps_main.tile([P, 512], DT.float32, tag="ps")
                for c in range(NCD):
                    nc.tensor.matmul(
                        ps_k[:], wk_t[:, c], h1g[:, c],
                        start=(c == 0), stop=(c == NCD - 1))
                with nc.allow_low_precision(reason="fp8 attention operand"):
                    nc.vector.tensor_copy(kt_sb[:, hp, ds(512 * g, 512)], ps_k[:])

            def v_unit(tb, g2, h1g):
                ps_v = ps_main.tile([P, WV], DT.float32, tag="ps")
                for c in range(NCD):
                    nc.tensor.matmul(
                        ps_v[:], h1g[:, c, ds(P * (tb % 4), P)],
                        wv_sb[:, c, ds(WV * g2, WV)],
                        start=(c == 0), stop=(c == NCD - 1))
                with nc.allow_low_precision(reason="fp8 attention operand"):
                    for hh in range(HPG):
                        h = HPG * g2 + hh
                        nc.vector.tensor_copy(
                            v_sb[:, tb, ds(65 * h, 64)],
                            ps_v[:, ds(64 * hh, 64)])

            def k_units(g, h1g):
                return [lambda hp=hp: k_unit(g, h1g, hp) for hp in range(NHP)]

            def v_units(tbs, h1g):
                return [lambda tb=tb, g2=g2: v_unit(tb, g2, h1g)
                        for tb in tbs for g2 in range(NVG)]

            # ---------------- attention superblock ----------------
            def attn_norm(mp, hp, po):
                rec = p_nrm.tile([1, 512], DT.float32r, tag="rec")
                with nc.allow_low_precision(reason="f32r broadcast operand"):
                    nc.vector.reciprocal(rec[:], po[64:65, :])
                pbc = ps_main.tile([P, 512], DT.float32, tag="ps", name="pbc")
                nc.tensor.matmul(pbc[0:64, :], ones_r[:, 0:64], rec[:],
                                 start=True, stop=True)
                rb = p_nrm.tile([64, 512], DT.float32, tag="rb")
                nc.vector.tensor_copy(rb[:], pbc[0:64, :])
                for hi in range(2):
                    nc.vector.tensor_mul(
                        ot_sb[64 * hi:64 * hi + 64, hp, ds(256 * mp, 256)],
                        po[0:64, ds(256 * hi, 256)], rb[:, ds(256 * hi, 256)])

            def attn(mp, fillers=()):
                nkv = 4 * mp + 4
                ngrp = nkv // 4
                scale = float(DK) ** -0.5
                L = len(fillers)
                pending = None
                for hp in range(NHP):
                    po = ps_av.tile([P, 512], DT.float32, tag="po")
                    for gk in range(ngrp):
                        band = gk == ngrp - 1
                        pss = [ps_s.tile([P, 1024], DT.float32, tag="ps_s",
                                         name=f"pss{_i}") for _i in range(2)]
                        for j in range(4):
                            kb = 4 * gk + j
                            for hi in range(2):
                                pb = 64 * hi
                                nc.tensor.matmul(
                                    pss[hi][:, ds(256 * j, 256)],
                                    kt_sb[pb:pb + 64, hp, ds(P * kb, P)],
                                    qt_sb[pb:pb + 64, hp, ds(256 * mp, 256)],
                                    start=(j % 2 == 0),
                                    stop=(j % 2 == 1) and not band,
                                    tile_position=(pb, 0),
                                    skip_group_check=True)
                        if band:
                            # fold the causal mask in additively on the PE:
                            # exp(-30000/8) underflows to exactly 0
                            for hi in range(2):
                                for s in range(2):
                                    nc.tensor.matmul(
                                        pss[hi][:, ds(512 * s, 512)],
                                        ident_sb[:],
                                        maskm[:, ds(512 * s, 512)],
                                        start=False, stop=True,
                                        skip_group_check=True)
                        exs = []
                        for hi in range(2):
                            ex = p_exp.tile([P, 2, 2, 256], F8, tag="ex")
                            with nc.allow_low_precision(reason="fp8 softmax wts"):
                                a = nc.scalar.activation(ex[:], pss[hi][:], AF.Exp,
                                                         bias=zero_b[:], scale=scale)
                            attn.last_exp = a.ins if hasattr(a, "ins") else a
                            exs.append(ex)
                        for pp in range(2):
                            for hi in range(2):
                                h = 2 * hp + hi
                                nc.tensor.matmul(
                                    po[0:65, ds(256 * hi, 256)],
                                    v_sb[:, ds(4 * gk + 2 * pp, 2), ds(65 * h, 65)],
                                    exs[hi][:, pp],
                                    start=(gk == 0 and pp == 0 and hi == 0),
                                    stop=(gk == ngrp - 1 and pp == 1 and hi == 1),
                                    perf_mode=mybir.MatmulPerfMode.DoubleRow,
                                    skip_group_check=True)
                    # norm chain runs one hp behind so it never heads the PE queue
                    if pending is not None:
                        attn_norm(mp, pending[0], pending[1])
                    pending = (hp, po)
                    for u in fillers[L * hp // NHP: L * (hp + 1) // NHP]:
                        u()
                attn_norm(mp, pending[0], pending[1])

            # ---------------- Wo + residual per group ----------------
            def c_unit(g, c):
                ps_x = ps_main.tile([P, 512], DT.float32, tag="ps")
                for k in range(NCD):
                    nc.tensor.matmul(
                        ps_x[:], wo_sb[:, k, ds(P * c, P)],
                        ot_sb[:, k, ds(512 * g, 512)],
                        start=(k == 0), stop=(k == NCD - 1))
                xq_t = p_xq.tile([P, 512], DT.float32, tag="xq")
                nc.sync.dma_start(xq_t[:], xqt_r[:, c, ds(512 * g, 512)])
                nc.vector.tensor_add(
                    x1_sb[:, c, ds(512 * g, 512)], ps_x[:], xq_t[:])

            def phase_C(g):
                for c in range(NCD):
                    c_unit(g, c)

            def c_units(g):
                return [lambda c=c: c_unit(g, c) for c in range(NCD)]

            # ---------------- LN2 per group ----------------
            def phase_D(g):
                sl = ds(512 * g, 512)
                ps1 = ps_main.tile([P, 512], DT.float32, tag="ps", name="ps1")
                for c in range(NCD):
                    nc.tensor.matmul(ps1[0:1, :], ones_bf[:], x1_sb[:, c, sl],
                                     start=(c == 0), stop=(c == NCD - 1))
                ps2 = ps_main.tile([P, 512], DT.float32, tag="ps", name="ps2")
                for c in range(NCD):
                    sq = p_sq.tile([P, 512], DT.bfloat16, tag="sq")
                    nc.scalar.activation(sq[:], x1_sb[:, c, sl], AF.Square,
                                         bias=zero_b[:])
                    nc.tensor.matmul(ps2[0:1, :], ones_bf[:], sq[:],
                                     start=(c == 0), stop=(c == NCD - 1))
                mu = p_st.tile([1, 512], DT.float32, tag="mu")
                nc.vector.tensor_scalar_mul(mu[:], ps1[0:1, :], 1.0 / D)
                msq = p_st.tile([1, 512], DT.float32, tag="msq")
                nc.vector.tensor_mul(msq[:], mu[:], mu[:])
                nc.vector.scalar_tensor_tensor(
                    msq[:], ps2[0:1, :], 1.0 / D, msq[:],
                    mybir.AluOpType.mult, mybir.AluOpType.subtract)
                # rstd = exp(-0.5 * ln(var + eps)) — stays on the exp/ln table set
                lnv = p_st.tile([1, 512], DT.float32, tag="lnv")
                nc.scalar.activation(lnv[:], msq[:], AF.Ln, bias=eps_b[:])
                r2f = p_st.tile([1, 512], DT.float32, tag="r2f")
                last_act = nc.scalar.activation(r2f[:], lnv[:], AF.Exp,
                                                bias=zero1[:], scale=-0.5)
                r2 = p_st.tile([1, 512], DT.float32r, tag="r2")
                with nc.allow_low_precision(reason="f32r broadcast operand"):
                    nc.vector.tensor_copy(r2[:], r2f[:])
                m2r = p_st.tile([1, 512], DT.float32r, tag="m2r")
                with nc.allow_low_precision(reason="f32r broadcast operand"):
                    nc.vector.tensor_mul(m2r[:], mu[:], r2[:])
                pb1 = ps_main.tile([P, 512], DT.float32, tag="ps", name="pb1")
                nc.tensor.matmul(pb1[:], ones_r[:], r2[:], start=True, stop=True)
                r2b = p_sq.tile([P, 512], DT.float32, tag="r2b")
                nc.vector.tensor_copy(r2b[:], pb1[:])
                pb2 = ps_main.tile([P, 512], DT.float32, tag="ps", name="pb2")
                nc.tensor.matmul(pb2[:], ones_r[:], m2r[:], start=True, stop=True)
                m2rb = p_sq.tile([P, 512], DT.float32, tag="m2rb")
                nc.vector.tensor_copy(m2rb[:], pb2[:])
                for c in range(NCD):
                    tmp = p_sq.tile([P, 512], DT.float32, tag="tmp")
                    nc.vector.tensor_mul(tmp[:], x1_sb[:, c, sl], r2b[:])
                    nc.vector.tensor_sub(xh_sb[:, c, sl], tmp[:], m2rb[:])
                return last_act.ins if hasattr(last_act, "ins") else last_act

            # ---------------- FFN per group ----------------
            def e_mm_chain(g1g, m, g):
                w1_t = p_w1s.tile([P, NCD, P], DT.bfloat16, tag="w1")
                nc.scalar.dma_start(w1_t[:], w1_r[:, :, ds(P * m, P)])
                ps_f = ps_main.tile([P, 512], DT.float32, tag="ps")
                for c in range(NCD):
                    nc.tensor.matmul(
                        ps_f[:], w1_t[:, c],
                        xh_sb[:, c, ds(512 * g, 512)],
                        start=(c == 0), stop=(c == NCD - 1))
                return ps_f

            def e_stage_units(g1g, g):
                # matmul + bf16 staging copy only; gelu deferred so these can
                # interleave with attention without ACT table thrash
                def unit(m):
                    ps_f = e_mm_chain(g1g, m, g)
                    nc.vector.tensor_copy(g1g[:, m], ps_f[:])
                return [lambda m=m: unit(m) for m in range(NCF)]

            def e_gelu_batch(g1g, dep_inst=None):
                for m in range(NCF):
                    a = nc.scalar.activation(g1g[:, m], g1g[:, m], AF.Gelu,
                                             bias=b1c_sb[:, ds(m, 1)])
                    if dep_inst is not None:
                        # every gelu must wait: the scheduler slots ACT ops by
                        # availability, and a gelu between exps costs two
                        # ~1.3us ACT-table loads
                        add_dep_helper(
                            a.ins if hasattr(a, "ins") else a, dep_inst,
                            sync=False, reason="gelu batch after attn exps")

            def phase_E(g, g1g):
                for m in range(NCF):
                    ps_f = e_mm_chain(g1g, m, g)
                    nc.scalar.activation(g1g[:, m], ps_f[:], AF.Gelu,
                                         bias=b1c_sb[:, ds(m, 1)])

            def phase_F(g, g1g, p_w2s):
                for c in range(NCD):
                    w2_t = p_w2s.tile([P, NCF, P], DT.bfloat16, tag="w2")
                    nc.sync.dma_start(w2_t[:], w2_r[:, :, ds(P * c, P)])
                    pf = ps_main.tile([P, 512], DT.float32, tag="ps")
                    for hh in range(NCF):
                        nc.tensor.matmul(pf[:], w2_t[:, hh], g1g[:, hh],
                                         start=(hh == 0), stop=(hh == NCF - 1))
                    out_t = p_out.tile([P, 512], DT.float32, tag="out")
                    nc.vector.tensor_add(out_t[:], pf[:],
                                         x1_sb[:, c, ds(512 * g, 512)])
                    nc.sync.dma_start(outt_r[:, c, ds(512 * g, 512)], out_t[:])

            # ---------------- emission schedule ----------------
            q_proj()
            aq_cm.__exit__(None, None, None)
            w1s_cm = tc.tile_pool(name="p_w1s", bufs=4, side="right")
            p_w1s = w1s_cm.__enter__()

            h1g0 = h1_load(0)
            for u in k_units(0, h1g0):
                u()
            for u in v_units(range(0, 4), h1g0):
                u()
            h1g1 = h1_load(1)
            attn(0, fillers=k_units(1, h1g1) + v_units(range(4, 8), h1g1))
            h1g2 = h1_load(2)
            attn(1, fillers=k_units(2, h1g2) + v_units(range(8, 12), h1g2))
            h1g3 = h1_load(3)
            attn(2, fillers=c_units(0) + [lambda: phase_D(0)]
                 + k_units(3, h1g3) + v_units(range(12, 16), h1g3))
            h1s_cm.__exit__(None, None, None)
            a_cm.__exit__(None, None, None)
            g1_cm = tc.tile_pool(name="p_g1", bufs=1, side="right")
            p_g1 = g1_cm.__enter__()
            g1g0 = p_g1.tile([P, NCF, 512], DT.bfloat16, tag="g1")
            attn(3, fillers=e_stage_units(g1g0, 0))
            last_exp3 = attn.last_exp
            phase_C(1)
            phase_D(1)

            qkv_cm.__exit__(None, None, None)
            w2s_cm = tc.tile_pool(name="p_w2s", bufs=3, side="left")
            p_w2s = w2s_cm.__enter__()

            e_gelu_batch(g1g0, last_exp3)
            phase_F(0, g1g0, p_w2s)
            g1g1 = p_g1.tile([P, NCF, 512], DT.bfloat16, tag="g1")
            phase_E(1, g1g1)
            phase_F(1, g1g1, p_w2s)

            w2s_cm.__exit__(None, None, None)
            g1_cm.__exit__(None, None, None)
            w1s_cm.__exit__(None, None, None)
            rep_cm.close()

    nc.compile()
    return nc


# ---------------- host side ----------------

def host_prep(x, Wq, Wk, Wv, Wo, bo, W1, b1, W2, b2, g1, be1, g2, be2):
    D = x.shape[2]
    H = Wq.shape[0]
    FF = W1.shape[1]
    NCF = FF // P
    f32 = np.float32

    mu = x.mean(-1, keepdims=True)
    var = ((x - mu) ** 2).mean(-1, keepdims=True)
    h1 = ((x - mu) / np.sqrt(var + EPS) * g1 + be1).astype(f32)

    shared = dict(
        wq=np.ascontiguousarray(Wq.transpose(1, 0, 2).reshape(D, -1)).astype(BF),
        wk=np.ascontiguousarray(Wk.transpose(1, 0, 2).reshape(D, -1)).astype(BF),
        wv=np.ascontiguousarray(Wv.transpose(1, 0, 2).reshape(D, -1)).astype(BF),
        wo=np.ascontiguousarray(Wo).astype(BF),
        w1=np.ascontiguousarray(g2[:, None] * W1).astype(BF),
        w2=np.ascontiguousarray(W2).astype(BF),
        b1c=np.ascontiguousarray((b1 + be2 @ W1).astype(f32).reshape(NCF, P).T),
        ident=np.eye(P, dtype=BF),
    )
    per_core = []
    for c in range(8):
        b, p = c // 2, c % 2
        r = np.arange(P)[:, None]
        j = np.arange(256)[None, :]
        qoff = np.where(j < P, 2 * j + p, 256 + 2 * (j - P) + p)
        m4 = np.zeros((P, 4, 256), f32)
        for t in range(4):
            m4[:, t, :] = (P * t + r <= qoff)
        m = np.concatenate([m4[:, 0::2, :], m4[:, 1::2, :]], axis=2)  # [P,2,512]
        per_core.append(dict(
            h1t=np.ascontiguousarray(h1[b].T).astype(BF),
            h1qt=np.ascontiguousarray(h1[b, p::2, :].T).astype(BF),
            xqt=np.ascontiguousarray(
                x[b, p::2, :].T + bo[:, None] + b2[:, None]).astype(f32),
            maskc=m.astype(BF),
            **shared,
        ))
    return per_core


def assemble(outts, B, T, D):
    out = np.zeros((B, T, D), np.float32)
    for c in range(8):
        b, p = c // 2, c % 2
        out[b, p::2, :] = outts[c].T
    return out


# ======================== top-level kernel entry ========================

_CACHE = {}


def _get_program():
    if "nc" not in _CACHE:
        _CACHE["nc"] = build_program(1024, 2048, 16, 64, 4096)
    return _CACHE["nc"]


def kernel(**inputs):
    """Full transformer block on 8 TRN2 NeuronCores.

    Takes the full unsharded inputs (as produced by setup_inputs()), shards
    (batch x query-parity) across 8 cores, runs the Bass SPMD kernel, and
    reassembles the full [4, 2048, 1024] float32 output.
    """
    from concourse.bass_utils import run_bass_kernel_spmd

    np_inputs = {k: np.asarray(v, np.float32) for k, v in inputs.items()}
    B, T, D = np_inputs["x"].shape
    nc = _get_program()
    per_core = host_prep(**np_inputs)
    res = run_bass_kernel_spmd(nc, per_core, list(range(8)))
    outts = [res.results[c]["outt"] for c in range(8)]
    return assemble(outts, B, T, D)
